# revision 42
# baseline (speedup 1.0000x reference)
"""Trainium2 Bass kernel for nn_BackProjLayer.

Math (validated vs reference, rel err ~1.4e-7):
  S is PSD by construction, so eigh + clamp(eigvals,0) + reconstruct == S
  up to fp32 noise.  Therefore:
    latent_raw[b,p] = Re(d_p^H S_b d_p)
    latent          = max(1e-6, BETA * tanh(latent_raw - tau))
    out[b]          = A diag(latent_b) A^H

Device layout (everything pixel-on-partitions, "transposed"):
  Stage 1:  TrT[p,(b,c)] = sum_e DD[e,p]  * SS[e,(b,c)]   (K=64 matmul)
            TiT[p,(b,c)] = sum_e DD2[e,p] * SS[e,(b,c)]
            where DD = [Dr;Di], DD2 = [Di;-Dr], SS[e,(b,c)] = [Sr_b;Si_b][e,c]
            Matmuls run in bf16x2 split precision (hi*hi + hi*lo + lo*hi
            accumulated in fp32 PSUM) — ~2^-17 per-product error, and the
            fused-fp32 LDWEIGHTS path is avoided (its single sync-wait slot
            cannot represent Tile's 2-wait matmuls).
            latT[p,b] = sum_c dT[p,c]*TrT[p,(b,c)] + dT[p,32+c]*TiT[p,(b,c)]
            (fp32 DVE multiply with broadcast AP + segmented tensor_reduce)
            latT = max(1e-6, BETA*tanh(latT - tau_p))  (tau = per-partition bias)
  Stage 2:  m1[p,(c,d)] = ArT[p,c]*ArT[p,d]   (GPSIMD, bf16 out)
            m2[p,(c,d)] = AiT[p,c]*AiT[p,d]
            m3[p,(c,d)] = AiT[p,c]*ArT[p,d]
            OutR[b,(c,d)] += latT^T @ m1 + latT^T @ m2   (PE, PSUM accum,
            partitions 0:64; the G = m1+m2 combine rides the accumulation)
            OI1[b,(c,d)]  += latT^T @ m3                 (partitions 64:128
            via tile_position=(0,64); host finishes OutI = OI1 - OI1^T,
            exploiting H = m3 - m3^T antisymmetry)
            (plain bf16 products: per-term 2e-3 errors average out over
             20000 pixels -> ~1e-5 of output scale)

Sharding: NPX=20000 split 8 ways (2500 px/core).  S replicated.  Host sums
the 8 partial [64,2048] outputs, antisymmetrizes OutI, reassembles latent.
"""

import os
from contextlib import ExitStack

import numpy as np

import concourse.bass as bass
import concourse.tile as tile
from concourse import bacc
from concourse import mybir
from concourse.bass_utils import run_bass_kernel_spmd

B, NCH, NPX, NCORES = 64, 32, 20000, 8
NP = NPX // NCORES  # 2500 pixels per core
BETA = float(1.0 / np.tanh(1.0))
F32 = mybir.dt.float32
BF16 = mybir.dt.bfloat16
CD = NCH * NCH  # 1024

# pixel tiles of 128 partitions (last one ragged: 2500 = 19*128 + 68)
PTILES = []
_p = 0
while _p < NP:
    PTILES.append((_p, min(128, NP - _p)))
    _p += 128
NT = len(PTILES)

_last_exec_time_ns = None
_last_in_maps = None
_cached = {}


def _split_hi_lo(nc, pool, src, name):
    """bf16x2 decomposition: hi = bf16(x), lo = bf16(x - hi)."""
    p, f = src.shape
    hi = pool.tile([p, f], BF16, name=f"{name}_hi")
    nc.vector.tensor_copy(hi[:, :], src[:, :])
    lo = pool.tile([p, f], BF16, name=f"{name}_lo")
    nc.vector.tensor_tensor(
        out=lo[:, :], in0=src[:, :], in1=hi[:, :], op=mybir.AluOpType.subtract
    )
    return hi, lo


def _build_nc():
    nc = bacc.Bacc(None, target_bir_lowering=False)
    dd = nc.declare_dram_parameter("dd", [2 * NCH, NP], F32, isOutput=False)
    dd2 = nc.declare_dram_parameter("dd2", [2 * NCH, NP], F32, isOutput=False)
    ss = nc.declare_dram_parameter("ss", [2 * NCH, B * NCH], F32, isOutput=False)
    dT = nc.declare_dram_parameter("dT", [NP, 2 * NCH], F32, isOutput=False)
    aT = nc.declare_dram_parameter("aT", [NP, 2 * NCH], F32, isOutput=False)
    tauv = nc.declare_dram_parameter("tauv", [NP, 1], F32, isOutput=False)
    latT = nc.declare_dram_parameter("latT", [NP, B], F32, isOutput=True)
    outp = nc.declare_dram_parameter("outp", [B, 2 * CD], F32, isOutput=True)

    with tile.TileContext(nc) as tc, ExitStack() as ctx:
        const = ctx.enter_context(tc.tile_pool(name="const", bufs=1))
        # Small per-tile tiles: one slot per pixel-tile so slots are never
        # recycled -> their DMAs/writes never wait on multi-proc releases
        # (HWDGE DMA descriptors have a single sync-wait slot).
        sba = ctx.enter_context(tc.tile_pool(name="sba", bufs=NT + 2))
        sbu = ctx.enter_context(tc.tile_pool(name="sbu", bufs=4))
        sbg = ctx.enter_context(tc.tile_pool(name="sbg", bufs=4))
        ps1 = ctx.enter_context(tc.tile_pool(name="ps1", bufs=3, space="PSUM"))
        pso = ctx.enter_context(tc.tile_pool(name="pso", bufs=1, space="PSUM"))

        # ---- one-time loads + bf16x2 splits ----
        dd_s = const.tile([2 * NCH, NP], F32)
        nc.sync.dma_start(out=dd_s[:, :], in_=dd[:, :])
        dd2_s = const.tile([2 * NCH, NP], F32)
        nc.sync.dma_start(out=dd2_s[:, :], in_=dd2[:, :])
        ss_s = const.tile([2 * NCH, B * NCH], F32)
        nc.sync.dma_start(out=ss_s[:, :], in_=ss[:, :])
        # dd2 arrives as [Di; Dr]; negate bottom half -> [Di; -Dr]
        nc.gpsimd.tensor_scalar_mul(dd2_s[NCH:, :], dd2_s[NCH:, :], -1.0)

        ss_hi, ss_lo = _split_hi_lo(nc, const, ss_s, "ss")
        dd_hi, dd_lo = _split_hi_lo(nc, const, dd_s, "dd")
        dd2_hi, dd2_lo = _split_hi_lo(nc, const, dd2_s, "dd2")

        outps = pso.tile([128, CD], F32)  # rows 0:64 OutR, 64:128 OI1 (2 banks)

        for ti, (p0, pn) in enumerate(PTILES):
            dT_t = sba.tile([128, 2 * NCH], F32, tag="dT_t")
            nc.sync.dma_start(out=dT_t[:pn, :], in_=dT[p0 : p0 + pn, :])
            aT_t = sba.tile([128, 2 * NCH], F32, tag="aT_t")
            nc.sync.dma_start(out=aT_t[:pn, :], in_=aT[p0 : p0 + pn, :])
            tau_t = sba.tile([128, 1], F32, tag="tau_t")
            nc.sync.dma_start(out=tau_t[:pn, :], in_=tauv[p0 : p0 + pn, :])
            ntau_t = sba.tile([128, 1], F32, tag="ntau_t")
            nc.gpsimd.tensor_scalar_mul(ntau_t[:pn, :], tau_t[:pn, :], -1.0)

            latT_t = sba.tile([128, B], F32, tag="latT_t")
            lraw = sba.tile([128, B], F32, tag="lraw")

            # ---- stage 1: two batch-halves of 32 ----
            for h in range(2):
                trh = ps1.tile([128, 32 * NCH], F32, tag="t12")  # 2 banks
                tih = ps1.tile([128, 32 * NCH], F32, tag="t12")  # 2 banks
                for psum, whi, wlo in ((trh, dd_hi, dd_lo), (tih, dd2_hi, dd2_lo)):
                    for i in range(2):
                        o = psum[:pn, 512 * i : 512 * (i + 1)]
                        r = slice(1024 * h + 512 * i, 1024 * h + 512 * (i + 1))
                        nc.tensor.matmul(
                            o, lhsT=whi[:, p0 : p0 + pn], rhs=ss_hi[:, r],
                            start=True, stop=False,
                        )
                        nc.tensor.matmul(
                            o, lhsT=whi[:, p0 : p0 + pn], rhs=ss_lo[:, r],
                            start=False, stop=False,
                        )
                        nc.tensor.matmul(
                            o, lhsT=wlo[:, p0 : p0 + pn], rhs=ss_hi[:, r],
                            start=False, stop=True,
                        )
                # u[p, b, c, 0] = TrT*Dr ; u[p, b, c, 1] = TiT*Di  (interleaved
                # writes; the add then rides the 2-axis XY tensor_reduce)
                u = sbu.tile([128, 2 * 32 * NCH], F32, tag="u")
                uv = u[:pn, :].rearrange("p (b c t) -> p b c t", c=NCH, t=2)
                drb = dT_t[:pn, 0:NCH].unsqueeze(1).broadcast_to((pn, 32, NCH))
                dib = dT_t[:pn, NCH:].unsqueeze(1).broadcast_to((pn, 32, NCH))
                nc.vector.tensor_tensor(
                    out=uv[:, :, :, 0:1].squeeze(3),
                    in0=trh[:pn, :].rearrange("p (b c) -> p b c", c=NCH),
                    in1=drb,
                    op=mybir.AluOpType.mult,
                )
                nc.vector.tensor_tensor(
                    out=uv[:, :, :, 1:2].squeeze(3),
                    in0=tih[:pn, :].rearrange("p (b c) -> p b c", c=NCH),
                    in1=dib,
                    op=mybir.AluOpType.mult,
                )
                nc.vector.tensor_reduce(
                    out=lraw[:pn, 32 * h : 32 * (h + 1)],
                    in_=uv,
                    axis=mybir.AxisListType.XY,
                    op=mybir.AluOpType.add,
                )

            th = sba.tile([128, B], F32, tag="th")
            nc.scalar.activation(
                th[:pn, :], lraw[:pn, :],
                mybir.ActivationFunctionType.Tanh,
                bias=ntau_t[:pn, 0:1], scale=1.0,
            )
            nc.vector.tensor_scalar(
                out=latT_t[:pn, :],
                in0=th[:pn, :],
                scalar1=BETA, scalar2=1e-6,
                op0=mybir.AluOpType.mult, op1=mybir.AluOpType.max,
            )
            nc.sync.dma_start(out=latT[p0 : p0 + pn, :], in_=latT_t[:pn, :])
            latT_bf = sba.tile([128, B], BF16, tag="latT_bf")
            nc.scalar.copy(latT_bf[:pn, :], latT_t[:pn, :])

            # ---- stage 2 pair products (bf16 outputs; no combine ops --
            # the PE accumulates m1+m2 into outR in PSUM, and the host does
            # OutI = OI1 - OI1^T since lam^T@(m3 - m3^T) = OI1 - OI1^T) ----
            arc = aT_t[:pn, 0:NCH].unsqueeze(2).broadcast_to((pn, NCH, NCH))
            ard = aT_t[:pn, 0:NCH].unsqueeze(1).broadcast_to((pn, NCH, NCH))
            aic = aT_t[:pn, NCH:].unsqueeze(2).broadcast_to((pn, NCH, NCH))
            aid = aT_t[:pn, NCH:].unsqueeze(1).broadcast_to((pn, NCH, NCH))
            m1 = sbg.tile([128, CD], BF16, tag="m1")
            m2 = sbg.tile([128, CD], BF16, tag="m2")
            m3 = sbg.tile([128, CD], BF16, tag="m3")
            v1 = m1[:pn, :].rearrange("p (c d) -> p c d", d=NCH)
            v2 = m2[:pn, :].rearrange("p (c d) -> p c d", d=NCH)
            v3 = m3[:pn, :].rearrange("p (c d) -> p c d", d=NCH)
            nc.gpsimd.tensor_tensor(out=v1, in0=arc, in1=ard, op=mybir.AluOpType.mult)
            nc.gpsimd.tensor_tensor(out=v2, in0=aic, in1=aid, op=mybir.AluOpType.mult)
            nc.gpsimd.tensor_tensor(out=v3, in0=aic, in1=ard, op=mybir.AluOpType.mult)

            # ---- stage 2 matmuls: accumulate into outps over tiles ----
            first, last = ti == 0, ti == NT - 1
            for i in range(2):
                o = outps[0:B, 512 * i : 512 * (i + 1)]
                nc.tensor.matmul(
                    o, lhsT=latT_bf[:pn, :], rhs=m1[:pn, 512 * i : 512 * (i + 1)],
                    start=first, stop=False,
                )
                nc.tensor.matmul(
                    o, lhsT=latT_bf[:pn, :], rhs=m2[:pn, 512 * i : 512 * (i + 1)],
                    start=False, stop=last,
                )
            for i in range(2):
                nc.tensor.matmul(
                    outps[B : 2 * B, 512 * i : 512 * (i + 1)],
                    lhsT=latT_bf[:pn, :],
                    rhs=m3[:pn, 512 * i : 512 * (i + 1)],
                    start=first, stop=last,
                    tile_position=(0, 64),
                )

        out_sb = const.tile([B, 2 * CD], F32)
        nc.scalar.copy(out_sb[:, 0:CD], outps[0:B, :])
        nc.scalar.copy(out_sb[:, CD : 2 * CD], outps[B : 2 * B, :])
        nc.sync.dma_start(out=outp[:, :], in_=out_sb[:, :])

    nc.compile()
    return nc


def kernel(S_real, S_imag, tau, D_real, D_imag, A_real, A_imag):
    global _last_exec_time_ns
    S_real = np.ascontiguousarray(S_real, np.float32)
    S_imag = np.ascontiguousarray(S_imag, np.float32)
    tau = np.ascontiguousarray(tau, np.float32)
    Dr = np.ascontiguousarray(D_real, np.float32)
    Di = np.ascontiguousarray(D_imag, np.float32)
    Ar = np.ascontiguousarray(A_real, np.float32)
    Ai = np.ascontiguousarray(A_imag, np.float32)

    Sr = S_real[:, 0]  # (B, NCH, NCH)
    Si = S_imag[:, 0]
    # SS[e,(b,c)] = [Sr_b; Si_b][e, c]
    ss = np.concatenate([Sr, Si], axis=1)  # [B, 64, 32]
    ss = np.ascontiguousarray(ss.transpose(1, 0, 2).reshape(2 * NCH, B * NCH))

    in_maps = []
    for g in range(NCORES):
        sl = slice(g * NP, (g + 1) * NP)
        dd_g = np.ascontiguousarray(np.concatenate([Dr[:, sl], Di[:, sl]], axis=0))
        dd2_g = np.ascontiguousarray(np.concatenate([Di[:, sl], Dr[:, sl]], axis=0))
        aT_g = np.ascontiguousarray(
            np.concatenate([Ar[:, sl], Ai[:, sl]], axis=0).T
        )
        in_maps.append(
            {
                "dd": dd_g,
                "dd2": dd2_g,
                "ss": ss,
                "dT": np.ascontiguousarray(dd_g.T),
                "aT": aT_g,
                "tauv": np.ascontiguousarray(tau[sl].reshape(NP, 1)),
            }
        )

    global _last_in_maps
    _last_in_maps = in_maps
    if "nc" not in _cached:
        _cached["nc"] = _build_nc()
    nc = _cached["nc"]

    trace = bool(int(os.environ.get("BPJ_TRACE", "0")))
    res = run_bass_kernel_spmd(nc, in_maps, list(range(NCORES)), trace=trace)
    _last_exec_time_ns = res.exec_time_ns

    lat_parts = [res.results[g]["latT"] for g in range(NCORES)]  # [NP, B] each
    latent = np.concatenate(lat_parts, axis=0).T.copy()  # [B, NPX]
    outp = np.sum([res.results[g]["outp"] for g in range(NCORES)], axis=0)
    outR = outp[:, :CD].reshape(B, NCH, NCH)
    oi1 = outp[:, CD:].reshape(B, NCH, NCH)
    outI = oi1 - oi1.transpose(0, 2, 1)
    out = (outR + 1j * outI).astype(np.complex64)
    return out, latent.astype(np.float32)
